# revision 32
# baseline (speedup 1.0000x reference)
"""Trainium2 Bass kernel for a ViT-style transformer block (pre-norm).

Strategy:
  - Pure data parallelism: 64 batches -> 8 per NeuronCore, no collectives.
  - Activations kept feature-major on device (xT: [D, tokens]) so every
    matmul contracts over the partition dimension with no transposes;
    the host transposes x on the way in and the output on the way out.
  - LayerNorm affine params and all biases are folded on the host into the
    adjacent weight matrices / bias vectors, so the device only computes
    the normalize step (x * alpha + beta with per-token alpha/beta).
  - LN mean / sum-of-squares via ones-vector matmuls (cross-partition
    reduction on the PE), with bf16 inputs so the chains run 1 cycle/row.
  - Attention per (batch, head) in scoresT layout ([key, query]); q/k/v
    and exp(scores) are bf16, so every attention matmul runs 1 cycle/row
    at the unpadded free size (197). The mask is appended as column DH of
    V, so the attn@V psum chain also yields the masked-softmax denominator
    (no separate sums matmuls).
  - Projection/FFN matmuls run in float32r (full PE speed at free >= 256).
  - FFN streams W1 tiles from HBM; W2 is loaded into the same SBUF buffer
    that held Wq/Wk/Wv/Wo during attention.
"""

import ml_dtypes
import numpy as np

import concourse.bacc as bacc
import concourse.mybir as mybir
from concourse.bass_utils import run_bass_kernel_spmd
from concourse.tile import TileContext

F32 = mybir.dt.float32
F32R = mybir.dt.float32r
BF16 = mybir.dt.bfloat16
AF = mybir.ActivationFunctionType
OP = mybir.AluOpType

N_CORES = 8
B, S, D, H, FF = 64, 197, 768, 12, 3072
DH = D // H  # 64
EPS = 1e-6
P = 128
CT = D // P  # 6 contraction tiles
FT = FF // P  # 24
GB = 2  # batches per group




def _ln_stats(
    nc, psA, sml, rep, sqpool, xt_g, ones_bf, N, tag_pfx, eps_sb,
    mm_tag="mm", mm_bufs=2, rep_tags=("a_rep", "b_rep"), rep_bufs=2,
    per_ct=False,
):
    """LN stats for one [128, CT, N] chunk -> broadcast alpha/beta tiles.

    Both reduction chains run in bf16 (1 cycle/row on the PE at any free
    size): a bf16 copy of x feeds the sum, a bf16 squares tile feeds the
    sum-of-squares. PSUM accumulates fp32, so only the input rounding
    (~0.4% per element, averaged over 768) touches the stats.
    """
    ps_sum = psA.tile([1, N], F32, tag=mm_tag, bufs=mm_bufs, name=f"{tag_pfx}_pssum")
    ps_sq = psA.tile([1, N], F32, tag=mm_tag, bufs=mm_bufs, name=f"{tag_pfx}_pssq")
    xb = sqpool.tile([P, CT, N], BF16, tag="xb", name=f"{tag_pfx}_xb")
    sq = sqpool.tile([P, CT, N], BF16, tag="sq", name=f"{tag_pfx}_sq")
    if not per_ct:
        nc.vector.tensor_scalar_mul(xb[:], xt_g[:], 1.0)
        nc.vector.tensor_mul(sq[:], xt_g[:], xt_g[:])
    for ct in range(CT):
        if per_ct:
            nc.vector.tensor_scalar_mul(xb[:, ct, :], xt_g[:, ct, :], 1.0)
            nc.vector.tensor_mul(sq[:, ct, :], xt_g[:, ct, :], xt_g[:, ct, :])
        nc.tensor.matmul(
            ps_sum[:], ones_bf[:, 0:1], xb[:, ct, :],
            start=(ct == 0), stop=(ct == CT - 1),
        )
        nc.tensor.matmul(
            ps_sq[:], ones_bf[:, 0:1], sq[:, ct, :],
            start=(ct == 0), stop=(ct == CT - 1),
        )
    return _ln_post(
        nc, sml, rep, ps_sum, ps_sq, N, tag_pfx, eps_sb, rep_tags, rep_bufs
    )


def _ln_post(nc, sml, rep, ps_sum, ps_sq, N, tag_pfx, eps_sb, rep_tags, rep_bufs):
    """Stats psums -> broadcast alpha/beta tiles.

    var and m are broadcast to all 128 partitions FIRST, so the sqrt and
    reciprocal run at full engine width (a [1,N] DVE reciprocal is
    single-channel and costs ~3us; the [128,N] approx version ~0.4us).
    """
    m = sml.tile([1, N], F32, tag="st_m", name=f"{tag_pfx}_m")
    msq = sml.tile([1, N], F32, tag="st_msq", name=f"{tag_pfx}_msq")
    var = sml.tile([1, N], F32, tag="st_var", name=f"{tag_pfx}_var")
    nc.vector.tensor_scalar_mul(m[:], ps_sum[:], 1.0 / D)
    nc.vector.tensor_scalar_mul(msq[:], ps_sq[:], 1.0 / D)
    nc.vector.tensor_mul(var[:], m[:], m[:])
    nc.vector.tensor_sub(var[:], msq[:], var[:])
    a_rep = rep.tile([P, N], F32, tag=rep_tags[0], bufs=rep_bufs, name=f"{tag_pfx}_arep")
    b_rep = rep.tile([P, N], F32, tag=rep_tags[1], bufs=rep_bufs, name=f"{tag_pfx}_brep")
    t_rep = rep.tile([P, N], F32, tag="ln_t", bufs=2, name=f"{tag_pfx}_trep")
    s_rep = rep.tile([P, N], F32, tag="ln_s", bufs=2, name=f"{tag_pfx}_srep")
    nc.gpsimd.partition_broadcast(t_rep[:], var[0:1, :])
    nc.scalar.activation(
        t_rep[:], t_rep[:], AF.Sqrt, bias=eps_sb[:, 0:1], scale=1.0
    )
    nc.vector.reciprocal_approx_accurate(
        out=a_rep[:], in_=t_rep[:], scratch=s_rep[:]
    )
    nc.gpsimd.partition_broadcast(b_rep[:], m[0:1, :])
    # beta = -(m * alpha)
    nc.vector.scalar_tensor_tensor(
        b_rep[:], b_rep[:], -1.0, a_rep[:], op0=OP.mult, op1=OP.mult
    )
    return a_rep, b_rep


def build_nc(n_cores=N_CORES, b_shard=8):
    """Build + compile the per-core kernel. b_shard = batches per core."""
    NG = b_shard // GB  # groups (= FFN chunks)
    T = b_shard * S  # tokens per core
    GT = GB * S  # tokens per group (394)

    nc = bacc.Bacc(
        "TRN2", target_bir_lowering=False, debug=False, num_devices=n_cores
    )

    xt_d = nc.dram_tensor("xt", [D, T], F32, kind="ExternalInput")
    wq_d = nc.dram_tensor("wq", [D, D], BF16, kind="ExternalInput")
    wk_d = nc.dram_tensor("wk", [D, D], BF16, kind="ExternalInput")
    wv_d = nc.dram_tensor("wv", [D, D], BF16, kind="ExternalInput")
    wo_d = nc.dram_tensor("wo", [D, D], BF16, kind="ExternalInput")
    w1_d = nc.dram_tensor("w1", [D, FF], BF16, kind="ExternalInput")
    w2_d = nc.dram_tensor("w2", [FF, D], BF16, kind="ExternalInput")
    bq_d = nc.dram_tensor("bq", [D], F32, kind="ExternalInput")
    bk_d = nc.dram_tensor("bk", [D], F32, kind="ExternalInput")
    bo_d = nc.dram_tensor("bo", [D], F32, kind="ExternalInput")
    b1_d = nc.dram_tensor("b1", [FF], F32, kind="ExternalInput")
    b2_d = nc.dram_tensor("b2", [D], F32, kind="ExternalInput")
    mk_d = nc.dram_tensor("mk", [P, 2 * b_shard], F32R, kind="ExternalInput")
    yt_d = nc.dram_tensor("yt", [D, T], F32, kind="ExternalOutput")

    def pon(ap_1d):  # [ (o p) ] -> [p, o]
        return ap_1d.rearrange("(o p) -> p o", p=P)

    def ponn(ap_2d):  # [(o p), n] -> [p, o, n]
        return ap_2d.rearrange("(o p) n -> p o n", p=P)

    with TileContext(nc) as tc:
        with (
            tc.tile_pool(name="const", bufs=1) as const,
            tc.tile_pool(name="xres", bufs=1) as xres,
            tc.tile_pool(name="sml", bufs=1) as sml,
            tc.tile_pool(name="rep", bufs=2) as rep,
            tc.tile_pool(name="sqp", bufs=2) as sqpool,
            tc.tile_pool(name="ffx", bufs=2) as ffx,
        ):
            # Resident weight buffers (bf16): Wq/Wk/Wv/Wo and W2.
            wbuf = const.tile([P, 4 * CT, D], BF16, tag="wbuf", name="wbuf")
            w2buf = const.tile([P, FT, D], BF16, tag="w2buf", name="w2buf")

            bq_sb = const.tile([P, CT], F32, tag="bq", name="bq_sb")
            bk_sb = const.tile([P, CT], F32, tag="bk", name="bk_sb")
            bo_sb = const.tile([P, CT], F32, tag="bo", name="bo_sb")
            b2_sb = const.tile([P, CT], F32, tag="b2", name="b2_sb")
            b1_sb = const.tile([P, FT], F32, tag="b1", name="b1_sb")
            mk_sb = const.tile([P, 2 * b_shard], F32R, tag="mk", name="mk_sb")
            ones = const.tile([P, 1], F32, tag="ones", name="ones_sb")
            ones_bf = const.tile([P, 1], BF16, tag="onesb", name="onesb_sb")
            eps_sb = const.tile([P, 1], F32, tag="eps", name="eps_sb")
            # Group 0's residual chunk lands first (split per-ct so LN1 can
            # begin after the first sixth), then the q/k weights.
            xg0 = xres.tile([P, CT, GT], F32, tag="xt0", name="xt0")
            for ct in range(CT):
                nc.sync.dma_start(
                    out=xg0[:, ct, :], in_=ponn(xt_d[:])[:, ct, 0:GT]
                )
            nc.vector.memset(eps_sb[:], EPS)
            nc.sync.dma_start(out=bq_sb[:], in_=pon(bq_d[:]))
            nc.sync.dma_start(out=bk_sb[:], in_=pon(bk_d[:]))
            nc.sync.dma_start(out=bo_sb[:], in_=pon(bo_d[:]))
            nc.sync.dma_start(out=b2_sb[:], in_=pon(b2_d[:]))
            nc.sync.dma_start(out=b1_sb[:], in_=pon(b1_d[:]))
            nc.sync.dma_start(out=mk_sb[:], in_=mk_d[:])
            nc.vector.memset(ones[:], 1.0)
            nc.vector.tensor_scalar_mul(ones_bf[:], ones[:], 1.0)

            # Residual stream, one tile per group/chunk. The first chunk and
            # the q/k weights land first so LN1/QKV of group 0 start early.
            xt_g = [xg0]
            for g in range(NG):
                if g > 0:
                    xg = xres.tile(
                        [P, CT, GT], F32, tag=f"xt{g}", name=f"xt{g}"
                    )
                    nc.sync.dma_start(
                        out=xg[:], in_=ponn(xt_d[:])[:, :, g * GT : (g + 1) * GT]
                    )
                    xt_g.append(xg)
                if g == 0:
                    nc.sync.dma_start(out=wbuf[:, 0:CT, :], in_=ponn(wq_d[:]))
                    nc.sync.dma_start(
                        out=wbuf[:, CT : 2 * CT, :], in_=ponn(wk_d[:])
                    )
                    nc.sync.dma_start(
                        out=wbuf[:, 2 * CT : 3 * CT, :], in_=ponn(wv_d[:])
                    )
                if g == min(1, NG - 1):
                    nc.sync.dma_start(
                        out=wbuf[:, 3 * CT : 4 * CT, :], in_=ponn(wo_d[:])
                    )
                if g == min(2, NG - 1):
                    nc.sync.dma_start(out=w2buf[:], in_=ponn(w2_d[:]))

            ln2_reps = []
            xh2_map = {}

            def emit_xh2(c):
                # Normalized FFN input for chunk c (DVE only).
                xgc = xt_g[c]
                a_rep, b_rep = ln2_reps[c]
                xh = ffx.tile(
                    [P, CT, GT], BF16, tag="xh2", bufs=2, name=f"xh2_{c}"
                )
                for ct in range(CT):
                    nc.vector.tensor_mul(xh[:, ct, :], xgc[:, ct, :], a_rep[:])
                    nc.vector.tensor_add(xh[:, ct, :], xh[:, ct, :], b_rep[:])
                return xh

            # ---------------- Phase A: attention ----------------
            with (
                tc.tile_pool(name="psA", bufs=1, space="PSUM") as psA,
                tc.tile_pool(name="psB", bufs=1, space="PSUM") as psB,
                tc.tile_pool(name="psC", bufs=1, space="PSUM") as psC,
                tc.tile_pool(name="attw", bufs=1) as attw,
                tc.tile_pool(name="attx", bufs=3) as attx,
            ):
                def emit_ln1(g):
                    return _ln_stats(
                        nc, psA, sml, rep, sqpool, xt_g[g], ones_bf, GT,
                        f"ln1g{g}", eps_sb, per_ct=(g == 0),
                    )

                def emit_xh(g, a_rep, b_rep):
                    # Normalized activations for group g (DVE only).
                    xg = xt_g[g]
                    xh = attw.tile(
                        [P, CT, GT], BF16, tag="xh", bufs=1, name=f"xh{g}"
                    )
                    for ct in range(CT):
                        nc.vector.tensor_mul(xh[:, ct, :], xg[:, ct, :], a_rep[:])
                        nc.vector.tensor_add(xh[:, ct, :], xh[:, ct, :], b_rep[:])
                    return xh

                def prep(g, xh):
                    # Q/K/V projections for group g.
                    xg = xt_g[g]
                    # Q/K in bf16: scores matmuls then run 1 cycle/row at the
                    # unpadded free size (197), so no 256-padding is needed.
                    qT = attw.tile(
                        [P, CT, GB, S], BF16, tag="qT", bufs=1, name=f"qT{g}"
                    )
                    kT = attw.tile(
                        [P, CT, GB, S], BF16, tag="kT", bufs=1, name=f"kT{g}"
                    )
                    for dst, wofs, bias in ((qT, 0, bq_sb), (kT, CT, bk_sb)):
                        for mt in range(CT):
                            ps = psA.tile(
                                [P, GT], F32, tag="mm", bufs=2,
                                name=f"psqk{g}_{wofs}_{mt}",
                            )
                            for ct in range(CT):
                                nc.tensor.matmul(
                                    ps[:],
                                    wbuf[:, wofs + ct, mt * P : (mt + 1) * P],
                                    xh[:, ct, :],
                                    start=(ct == 0), stop=(ct == CT - 1),
                                )
                            nc.scalar.activation(
                                dst[:, mt, :, :],
                                ps[:].rearrange("p (b s) -> p b s", b=GB),
                                AF.Identity,
                                bias=bias[:, mt : mt + 1], scale=1.0,
                            )

                    # V in token-major layout (bf16), rows scaled by the
                    # attention mask; column DH holds the mask itself so the
                    # attn@V matmul chain also emits the masked-softmax
                    # denominator in psum row DH.
                    vT = attw.tile(
                        [P, GB, 2, H, DH + 1], BF16, tag="vT", bufs=1,
                        name=f"vT{g}"
                    )
                    for b2 in range(GB):
                        for tt in range(2):
                            off = b2 * S + tt * P
                            M = P if tt == 0 else S - P
                            mi = (g * GB + b2) * 2 + tt
                            for hf in range(2):
                                ps = psA.tile(
                                    [P, D // 2], F32, tag="mm", bufs=2,
                                    name=f"psv{g}_{b2}_{tt}_{hf}",
                                )
                                for ct in range(CT):
                                    nc.tensor.matmul(
                                        ps[:M, :],
                                        xh[:, ct, off : off + M],
                                        wbuf[
                                            :, 2 * CT + ct,
                                            hf * (D // 2) : (hf + 1) * (D // 2),
                                        ],
                                        start=(ct == 0), stop=(ct == CT - 1),
                                    )
                                nc.scalar.activation(
                                    vT[
                                        0:M, b2, tt,
                                        hf * (H // 2) : (hf + 1) * (H // 2),
                                        0:DH,
                                    ],
                                    ps[:M, :].rearrange("p (h d) -> p h d", h=H // 2),
                                    AF.Identity,
                                    scale=mk_sb[0:M, mi : mi + 1].bitcast(F32),
                                )
                            nc.vector.tensor_scalar_mul(
                                vT[0:M, b2, tt, :, DH : DH + 1],
                                ones[0:M, 0:1].to_broadcast((M, H, 1)),
                                mk_sb[0:M, mi : mi + 1].bitcast(F32),
                            )
                    return qT, kT, vT

                ln1_reps = {0: emit_ln1(0)}
                xh_map = {0: emit_xh(0, *ln1_reps[0])}
                for g in range(NG):
                    qT, kT, vT = prep(g, xh_map[g])
                    xg = xt_g[g]
                    # LN1 stats for the NEXT group go on the PE queue here so
                    # their DVE/gpsimd post-processing (alpha/beta) runs while
                    # the PE is busy with this group's attention.
                    if g + 1 < NG:
                        ln1_reps[g + 1] = emit_ln1(g + 1)

                    attnT = attw.tile([P, CT, GT], BF16, tag="attnT", name=f"at{g}")
                    for b2 in range(GB):
                        ps_pair = None
                        for h in range(H):
                            hp, rh = h // 2, (h % 2) * DH
                            ps_sc = psB.tile(
                                [P, 2, S], F32, tag="sc", bufs=3,
                                name=f"s_{g}{b2}{h}",
                            )
                            nc.tensor.matmul(
                                ps_sc[:, 0, :],
                                kT[rh : rh + DH, hp, b2, 0:P],
                                qT[rh : rh + DH, hp, b2, :],
                                start=True, stop=True,
                            )
                            nc.tensor.matmul(
                                ps_sc[0 : S - P, 1, :],
                                kT[rh : rh + DH, hp, b2, P:S],
                                qT[rh : rh + DH, hp, b2, :],
                                start=True, stop=True,
                            )
                            expT = attx.tile(
                                [P, 2, S], BF16, tag="exp", name=f"e_{g}{b2}{h}"
                            )
                            # One exp over the whole psum tile: rows 69:128 of
                            # slot 1 are stale-but-finite psum (scores of an
                            # earlier head) and are never consumed downstream.
                            nc.scalar.activation(
                                expT[:], ps_sc[:], AF.Exp, scale=1.0
                            )
                            if h % 2 == 0:
                                ps_pair = psC.tile(
                                    [DH + 1, 2, S], F32, tag="at", bufs=3,
                                    name=f"a_{g}{b2}{h}",
                                )
                            ps_a = ps_pair[:, h % 2, :]
                            nc.tensor.matmul(
                                ps_a,
                                vT[:, b2, 0, h, :],
                                expT[:, 0, :],
                                start=True, stop=False,
                            )
                            nc.tensor.matmul(
                                ps_a,
                                vT[0 : S - P, b2, 1, h, :],
                                expT[0 : S - P, 1, :],
                                start=False, stop=True,
                            )
                            if h % 2 == 1:
                                # Both heads of the pair done: one staging
                                # copy of the two denominator rows (psum row
                                # DH, [1, 2*S]; DVE reads above partition 0
                                # must be plain ops), one pair reciprocal,
                                # one pair broadcast, two normalizing muls.
                                r_rep = attx.tile(
                                    [DH, 2, S], F32, tag="rrep",
                                    name=f"rr_{g}{b2}{h}",
                                )
                                r_scr = attx.tile(
                                    [1, 2, S], F32, tag="rscr",
                                    name=f"rs_{g}{b2}{h}",
                                )
                                r_den = attx.tile(
                                    [1, 2, S], F32, tag="rden",
                                    name=f"rd_{g}{b2}{h}",
                                )
                                nc.vector.tensor_scalar_mul(
                                    r_den[0:1, :, :],
                                    ps_pair[DH : DH + 1, :, :], 1.0,
                                )
                                nc.vector.reciprocal_approx_accurate(
                                    out=r_rep[0:1, :, :],
                                    in_=r_den[0:1, :, :],
                                    scratch=r_scr[0:1, :, :],
                                )
                                nc.gpsimd.partition_broadcast(
                                    r_rep[:, :, :], r_rep[0:1, :, :]
                                )
                                for par in range(2):
                                    hh = h - 1 + par
                                    rh2 = (hh % 2) * DH
                                    nc.vector.tensor_mul(
                                        attnT[
                                            rh2 : rh2 + DH, hh // 2,
                                            b2 * S : (b2 + 1) * S,
                                        ],
                                        ps_pair[0:DH, par, :],
                                        r_rep[:, par, :],
                                    )

                    # xh for the next group goes on the DVE queue here —
                    # ahead of this group's residual adds (which wait on the
                    # out-proj psums) — so it completes during attention and
                    # QKV(g+1) starts stall-free.
                    if g + 1 < NG:
                        xh_map[g + 1] = emit_xh(g + 1, *ln1_reps[g + 1])

                    # Output projection + residual (in place into xg), with
                    # the LN2 stat chains interleaved per-mt so the PE never
                    # waits for a whole-tile xb/sq pass. The LN2 chains live
                    # in psB (free during out-proj) to stay within 8 banks.
                    ps_sum2 = psB.tile(
                        [1, GT], F32, tag="sc", bufs=3, name=f"ln2s{g}"
                    )
                    ps_sq2 = psB.tile(
                        [1, GT], F32, tag="sc", bufs=3, name=f"ln2q{g}"
                    )
                    xb2 = sqpool.tile(
                        [P, CT, GT], BF16, tag="xb", name=f"ln2xb{g}"
                    )
                    sq2 = sqpool.tile(
                        [P, CT, GT], BF16, tag="sq", name=f"ln2sq{g}"
                    )
                    for mt in range(CT):
                        ps = psA.tile(
                            [P, GT], F32, tag="mm", bufs=2, name=f"pso{g}_{mt}"
                        )
                        for ct in range(CT):
                            nc.tensor.matmul(
                                ps[:],
                                wbuf[:, 3 * CT + ct, mt * P : (mt + 1) * P],
                                attnT[:, ct, :],
                                start=(ct == 0), stop=(ct == CT - 1),
                            )
                        nc.vector.scalar_tensor_tensor(
                            xg[:, mt, :], ps[:], bo_sb[:, mt : mt + 1],
                            xg[:, mt, :], op0=OP.add, op1=OP.add,
                        )
                        nc.vector.tensor_scalar_mul(
                            xb2[:, mt, :], xg[:, mt, :], 1.0
                        )
                        nc.vector.tensor_mul(
                            sq2[:, mt, :], xg[:, mt, :], xg[:, mt, :]
                        )
                        nc.tensor.matmul(
                            ps_sum2[:], ones_bf[:, 0:1], xb2[:, mt, :],
                            start=(mt == 0), stop=(mt == CT - 1),
                        )
                        nc.tensor.matmul(
                            ps_sq2[:], ones_bf[:, 0:1], sq2[:, mt, :],
                            start=(mt == 0), stop=(mt == CT - 1),
                        )
                    ln2_reps.append(
                        _ln_post(
                            nc, sml, rep, ps_sum2, ps_sq2, GT, f"ln2c{g}",
                            eps_sb, ("a2_rep", "b2_rep"), NG,
                        )
                    )
                    if g == 0:
                        # FFN chunk 0's xhat can be computed here already —
                        # removes the Phase A -> B serial DVE dependency.
                        xh2_map[0] = emit_xh2(0)

            # ---------------- Phase B: FFN ----------------
            with (
                tc.tile_pool(name="psU", bufs=1, space="PSUM") as psU,
                tc.tile_pool(name="psY", bufs=1, space="PSUM") as psY,
                tc.tile_pool(name="ffw", bufs=1) as ffw,
            ):
                for c in range(NG):
                    xg = xt_g[c]
                    xh = xh2_map[c]
                    ps_y = [
                        psY.tile([P, GT], F32, tag=f"y{mt}", name=f"psy{c}_{mt}")
                        for mt in range(CT)
                    ]
                    for ft in range(FT):
                        w1t = ffw.tile(
                            [P, CT, P], BF16, tag="w1", bufs=3, name=f"w1_{c}_{ft}"
                        )
                        nc.sync.dma_start(
                            out=w1t[:], in_=ponn(w1_d[:])[:, :, ft * P : (ft + 1) * P]
                        )
                        ps_u = psU.tile(
                            [P, GT], F32, tag="st_sum", bufs=2, name=f"psu{c}_{ft}"
                        )
                        for ct in range(CT):
                            nc.tensor.matmul(
                                ps_u[:],
                                w1t[:, ct, :],
                                xh[:, ct, :],
                                start=(ct == 0), stop=(ct == CT - 1),
                            )
                        g_sb = ffw.tile([P, GT], BF16, tag="g", bufs=3, name=f"g{c}_{ft}")
                        nc.scalar.activation(
                            g_sb[:], ps_u[:], AF.Gelu,
                            bias=b1_sb[:, ft : ft + 1], scale=1.0,
                        )
                        for mt in range(CT):
                            nc.tensor.matmul(
                                ps_y[mt][:],
                                w2buf[:, ft, mt * P : (mt + 1) * P],
                                g_sb[:],
                                start=(ft == 0), stop=(ft == FT - 1),
                            )
                    # Next chunk's xhat goes on the DVE queue ahead of this
                    # chunk's residual adds (which wait on late fc2 psums).
                    if c + 1 < NG:
                        xh2_map[c + 1] = emit_xh2(c + 1)
                    for mt in range(CT):
                        nc.vector.scalar_tensor_tensor(
                            xg[:, mt, :], ps_y[mt][:], b2_sb[:, mt : mt + 1],
                            xg[:, mt, :], op0=OP.add, op1=OP.add,
                        )
                        nc.sync.dma_start(
                            out=ponn(yt_d[:])[:, mt, c * GT : (c + 1) * GT],
                            in_=xg[:, mt, :],
                        )

    nc.compile()
    return nc


def to_bf16(a):
    return np.ascontiguousarray(a, np.float32).astype(ml_dtypes.bfloat16)


def to_fp32r(a):
    """Round fp32 -> fp32r (e8m11, round-to-nearest-even), keep fp32 layout."""
    u = np.ascontiguousarray(a, np.float32).view(np.uint32)
    r = (u + np.uint32(0x7FF) + ((u >> np.uint32(12)) & np.uint32(1))) & np.uint32(
        0xFFFFF000
    )
    return r.view(np.float32)


def host_prep(inputs, b_shard=8):
    """Fold LN affine + biases into weights; build per-core input maps."""
    f = np.float32
    x = np.ascontiguousarray(inputs["x"], dtype=f)
    Wq, bq = np.asarray(inputs["Wq"], f), np.asarray(inputs["bq"], f)
    Wk, bk = np.asarray(inputs["Wk"], f), np.asarray(inputs["bk"], f)
    Wv, bv = np.asarray(inputs["Wv"], f), np.asarray(inputs["bv"], f)
    Wo, bo = np.asarray(inputs["Wo"], f), np.asarray(inputs["bo"], f)
    W1, b1 = np.asarray(inputs["W1"], f), np.asarray(inputs["b1"], f)
    W2, b2 = np.asarray(inputs["W2"], f), np.asarray(inputs["b2"], f)
    ln1w, ln1b = np.asarray(inputs["ln1_w"], f), np.asarray(inputs["ln1_b"], f)
    ln2w, ln2b = np.asarray(inputs["ln2_w"], f), np.asarray(inputs["ln2_b"], f)
    mask = np.asarray(inputs["mask"])

    s = f(1.0 / np.sqrt(DH))
    wq_e = np.ascontiguousarray((ln1w[:, None] * Wq) * s)
    bq_e = (ln1b @ Wq + bq) * s
    wk_e = np.ascontiguousarray(ln1w[:, None] * Wk)
    bk_e = ln1b @ Wk + bk
    wv_e = np.ascontiguousarray(ln1w[:, None] * Wv)
    bv_e = ln1b @ Wv + bv
    bo_e = bv_e @ Wo + bo
    w1_e = np.ascontiguousarray(ln2w[:, None] * W1)
    b1_e = ln2b @ W1 + b1

    mask_f = mask.astype(f)  # [B, S]

    n_cores = B // b_shard
    in_maps = []
    for c in range(n_cores):
        xs = x[c * b_shard : (c + 1) * b_shard]  # [b_shard, S, D]
        xt = np.ascontiguousarray(
            xs.transpose(2, 0, 1).reshape(D, b_shard * S)
        )
        mk = np.zeros((P, 2 * b_shard), f)
        ms = mask_f[c * b_shard : (c + 1) * b_shard]  # [b_shard, S]
        for b_ in range(b_shard):
            mk[:, 2 * b_] = ms[b_, 0:P]
            mk[0 : S - P, 2 * b_ + 1] = ms[b_, P:S]
        in_maps.append(
            {
                "xt": xt,
                "wq": to_bf16(wq_e), "wk": to_bf16(wk_e),
                "wv": to_bf16(wv_e), "wo": to_bf16(Wo),
                "w1": to_bf16(w1_e), "w2": to_bf16(W2),
                "bq": bq_e, "bk": bk_e, "bo": bo_e,
                "b1": b1_e, "b2": b2, "mk": mk,
            }
        )
    return in_maps


_NC_CACHE = {}


def get_nc(n_cores=N_CORES, b_shard=8):
    key = (n_cores, b_shard)
    if key not in _NC_CACHE:
        _NC_CACHE[key] = build_nc(n_cores, b_shard)
    return _NC_CACHE[key]


def kernel(**inputs):
    b_shard = B // N_CORES
    nc = get_nc(N_CORES, b_shard)
    in_maps = host_prep(inputs, b_shard)
    res = run_bass_kernel_spmd(nc, in_maps, list(range(N_CORES)))
    outs = []
    for c in range(N_CORES):
        yt = res.results[c]["yt"]  # [D, b_shard*S]
        outs.append(yt.reshape(D, b_shard, S).transpose(1, 2, 0))
    return np.ascontiguousarray(np.concatenate(outs, axis=0), dtype=np.float32)

